# revision 32
# baseline (speedup 1.0000x reference)
"""Two-level VQ assignment kernel for Trainium2 (8 NeuronCores, data-parallel).

Per core: NBLK blocks of 1024 samples (8 tiles of 128). Stage-1 scores
s1 = 2*x1.c - |c|^2 + j*eps come from x-as-stationary matmuls with a bf16
hi/lo split (bias triple-split so the j*eps tie-break survives rounding).
The winner is found with one grouped reduce_max + one is_equal per block.
One-hot masks are pair-transposed on the PE; stage-2 then runs almost
entirely on the PE: a small paired gather pulls each sample's 8 inner-center
norms (+ an 8j id column) into PSUM, a constant-stationary matmul gathers
the transposed coefficients ccT[(k,d), s] = 2*ci[outer(s),k,d], a pattern
matmul replicates x2 over the 8 k-groups, one 2x-rate bf16 multiply forms
the products, and per-tile fold matmuls (prodsT^T @ F) accumulate the
17-term dots straight onto the norm PSUM. The inner argmax runs on the
[128, 8, 8] result; id = 8*outer + inner lands in a u16 staging buffer
DMA'd out once at the end.
"""

import sys

sys.path.insert(0, "/opt/trn_rl_repo")

import numpy as np
import ml_dtypes

import concourse.bass as bass
import concourse.bacc as bacc
import concourse.mybir as mybir
from concourse import tile
from concourse.bass_types import AP
import jax
from jax.sharding import Mesh, PartitionSpec
from jax.experimental.shard_map import shard_map

N = 1_000_000
D1, D2 = 16, 16
NC, NCPC = 64, 8
NCORES = 8
NPC = N // NCORES          # real samples per core (125000)
P = 128                    # partition tile
TPB = 8                    # tiles per block
BS = P * TPB               # samples per block (1024)
XB = 4                     # blocks per input DMA
NBLK = XB * ((NPC + XB * BS - 1) // (XB * BS))
NBX = NBLK // XB
NPB = NBLK * BS            # padded per-core sample count
EPS1 = 2.0 ** -16          # stage-1 tie-break increment per cluster id
BF16 = mybir.dt.bfloat16
F32 = mybir.dt.float32
U16 = mybir.dt.uint16

_cache = {}
TRACE = False
import os as _os
KSTAGE = int(_os.environ.get("KSTAGE", "5"))
# 1: s1+argmax+transposes+dummy out; 2: +ccT gather; 3: +cc9 norm gather;
# 4: +mult+folds; 5: full

AX = mybir.AxisListType
OP = mybir.AluOpType
AF = mybir.ActivationFunctionType


def _build_kernel(reps=1):
    nc = bacc.Bacc()
    xs = nc.dram_tensor("xs", [NBX * 35, XB * BS], BF16, kind="ExternalInput")
    x2r8 = nc.dram_tensor("x2r8", [NBX * 128, XB * BS], BF16, kind="ExternalInput")
    w1 = nc.dram_tensor("w1", [35, 128], BF16, kind="ExternalInput")
    tabn = nc.dram_tensor("tabn", [64, 9], BF16, kind="ExternalInput")
    tabcT = nc.dram_tensor("tabcT", [128, 128], BF16, kind="ExternalInput")
    foldF = nc.dram_tensor("foldF", [128, 9], BF16, kind="ExternalInput")
    ident = nc.dram_tensor("ident", [128, 128], BF16, kind="ExternalInput")
    iotak = nc.dram_tensor("iotak", [128, 8], BF16, kind="ExternalInput")
    out = nc.dram_tensor("out", [P, NBLK * TPB], U16, kind="ExternalOutput")

    with tile.TileContext(nc) as tc:
        with (
            tc.tile_pool(name="const", bufs=1) as cpool,
            tc.tile_pool(name="xin", bufs=3) as xpool,
            tc.tile_pool(name="x2in", bufs=3) as x2pool,
            tc.tile_pool(name="s1ps", bufs=2, space="PSUM") as s1pool,
            tc.tile_pool(name="trps", bufs=2, space="PSUM") as trpool,
            tc.tile_pool(name="ccTps", bufs=1, space="PSUM") as ccTpool,
            tc.tile_pool(name="cc9ps", bufs=2, space="PSUM") as cc9pool,
            tc.tile_pool(name="work", bufs=2) as wpool,
        ):
            w1sb = cpool.tile([35, 128], BF16)
            nc.sync.dma_start(w1sb[:], w1[:])
            tabnsb = cpool.tile([64, 9], BF16)
            nc.sync.dma_start(tabnsb[:], tabn[:])
            tabcsb = cpool.tile([128, 128], BF16)
            nc.sync.dma_start(tabcsb[:], tabcT[:])
            foldsb = cpool.tile([128, 9], BF16)
            nc.sync.dma_start(foldsb[:], foldF[:])
            idsb = cpool.tile([128, 128], BF16)
            nc.sync.dma_start(idsb[:], ident[:])
            ioksb = cpool.tile([128, 8], BF16)
            nc.sync.dma_start(ioksb[:], iotak[:])
            outbuf = cpool.tile([P, NBLK * TPB], U16)

            for rep_b in range(reps * NBLK):
                b = rep_b % NBLK
                if b % XB == 0:
                    bx = b // XB
                    xt4 = xpool.tile([35, XB, BS], BF16, tag="xs")
                    nc.sync.dma_start(xt4[:], xs[bx * 35:(bx + 1) * 35, :])
                    x2t4 = x2pool.tile([P, XB, BS], BF16, tag="x2")
                    # spread the 1MB replicated-x2 load over 3 DGE queues
                    for (eng, lo, hi) in ((nc.sync, 0, 48),
                                          (nc.scalar, 48, 96),
                                          (nc.gpsimd, 96, 128)):
                        eng.dma_start(
                            x2t4[lo:hi, :, :],
                            x2r8[bx * P + lo:bx * P + hi, :])
                xt = xt4[:, b % XB, :]
                x2rs = x2t4[:, b % XB, :]

                if KSTAGE in (-1, -2):
                    if b % XB == 0:
                        xr = AP(xt4.tensor, xt4[:].offset,
                                [[xt4[:].ap[0][0], 32], [1, TPB]])
                        x2r = AP(x2t4.tensor, x2t4[:].offset,
                                 [[x2t4[:].ap[0][0], 32], [1, TPB]])
                        nc.vector.scalar_tensor_tensor(
                            out=outbuf[0:32, b * TPB:(b + 1) * TPB],
                            in0=xr, scalar=1.0,
                            in1=(x2r if KSTAGE == -1 else xr),
                            op0=OP.mult, op1=OP.add)
                    continue

                # stage-1 scores for the whole block into one PSUM bank
                s1 = s1pool.tile([P, TPB, NC], F32, tag="s1")
                for t in range(TPB):
                    xsl = xt[:, t * P:(t + 1) * P]
                    nc.tensor.matmul(s1[:, t, :], xsl, w1sb[:, 0:64],
                                     start=True, stop=False)
                    nc.tensor.matmul(s1[:, t, :], xsl, w1sb[:, 64:128],
                                     start=False, stop=True)

                # per-sample winner value, then exact-match one-hot
                w = wpool.tile([P, TPB], F32, tag="w")
                nc.vector.tensor_reduce(w[:], s1[:], axis=AX.X, op=OP.max)
                oh = wpool.tile([P, TPB, NC], BF16, tag="oh")
                wb = AP(w.tensor, w[:].offset,
                        [w[:].ap[0], [1, TPB], [0, NC]])
                nc.vector.tensor_tensor(out=oh[:], in0=s1[:], in1=wb,
                                        op=OP.is_equal)

                # transpose one-hots: [128, 64] -> [64, 128] per tile
                # (base-64 matmul operands crash the exec unit, so each
                # tile's ohT must live on partitions 0:63)
                ohtr = trpool.tile([64, TPB, P], BF16, tag="ohtr")
                for t in range(TPB):
                    nc.tensor.transpose(ohtr[:, t, :], oh[:, t, :], idsb[:])
                ohts = wpool.tile([64, TPB, P], BF16, tag="ohts")
                nc.scalar.activation(ohts[:], ohtr[:], AF.Copy)

                if KSTAGE < 2:
                    in1w = w[:] if KSTAGE == 1 else AP(
                        x2t4.tensor, x2t4[:].offset + (b % XB) * BS,
                        [x2t4[:].ap[0], [1, TPB]])
                    nc.vector.scalar_tensor_tensor(
                        out=outbuf[:, b * TPB:(b + 1) * TPB],
                        in0=w[:], scalar=1.0, in1=in1w,
                        op0=OP.mult, op1=OP.bypass)
                    continue
                # transposed coefficient gather: ccT[(k,d), s]
                ccT = ccTpool.tile([P, TPB, P], F32, tag="ccT")
                for t in range(TPB):
                    nc.tensor.matmul(ccT[:, t, :], tabcsb[0:64, :],
                                     ohts[:, t, :], start=True, stop=True)
                if KSTAGE == 25:
                    with nc.allow_low_precision(reason="debug"):
                        nc.vector.tensor_reduce(
                            outbuf[:, b * TPB:(b + 1) * TPB], ccT[:, :, 0:1],
                            axis=AX.X, op=OP.add)
                    continue
                ccTs = wpool.tile([P, TPB, P], BF16, tag="ccTs")
                nc.scalar.activation(ccTs[:], ccT[:], AF.Copy)

                if KSTAGE < 3:
                    with nc.allow_low_precision(reason="debug"):
                        nc.vector.tensor_reduce(
                            outbuf[:, b * TPB:(b + 1) * TPB], ccTs[:, :, 0:1],
                            axis=AX.X, op=OP.add)
                    continue
                # norm gather (+8j id col): cc9[s, t, 0:8] = -|ci|^2,
                # col 8 = 8*outer; fold-mms accumulate the dots on top
                cc9 = cc9pool.tile([P, TPB, 9], F32, tag="cc9")
                for t in range(TPB):
                    nc.tensor.matmul(
                        AP(cc9.tensor, cc9[:].offset + t * 9,
                           [cc9[:].ap[0], [1, 9]]),
                        ohts[:, t, :], tabnsb[:],
                        start=(t == 0), stop=False, skip_group_check=True)

                if KSTAGE < 4:
                    jv0 = AP(cc9.tensor, cc9[:].offset + 8,
                             [cc9[:].ap[0], [9, TPB]])
                    nc.vector.scalar_tensor_tensor(
                        out=outbuf[:, b * TPB:(b + 1) * TPB],
                        in0=jv0, scalar=1.0, in1=jv0,
                        op0=OP.mult, op1=OP.bypass)
                    continue
                prodsT = wpool.tile([P, TPB, P], BF16, tag="prodsT")
                x2v = AP(x2rs.tensor, x2rs.offset,
                         [x2rs.ap[0], [P, TPB], [1, P]])
                nc.vector.tensor_tensor(out=prodsT[:], in0=ccTs[:],
                                        in1=x2v, op=OP.mult)
                for t in range(TPB):
                    nc.tensor.matmul(
                        AP(cc9.tensor, cc9[:].offset + t * 9,
                           [cc9[:].ap[0], [1, 9]]),
                        prodsT[:, t, :], foldsb[:],
                        start=False, stop=(t == TPB - 1),
                        skip_group_check=True)

                if KSTAGE < 5:
                    jv0 = AP(cc9.tensor, cc9[:].offset + 8,
                             [cc9[:].ap[0], [9, TPB]])
                    nc.vector.scalar_tensor_tensor(
                        out=outbuf[:, b * TPB:(b + 1) * TPB],
                        in0=jv0, scalar=1.0, in1=jv0,
                        op0=OP.mult, op1=OP.bypass)
                    continue
                # inner argmax via grouped max + one-hot dot iota_k
                u2v = AP(cc9.tensor, cc9[:].offset,
                         [cc9[:].ap[0], [9, TPB], [1, NCPC]])
                w2 = wpool.tile([P, TPB], F32, tag="w2")
                nc.vector.tensor_reduce(w2[:], u2v, axis=AX.X, op=OP.max)
                oh2 = wpool.tile([P, TPB, NCPC], BF16, tag="oh2")
                w2b = AP(w2.tensor, w2[:].offset,
                         [w2[:].ap[0], [1, TPB], [0, NCPC]])
                nc.vector.tensor_tensor(out=oh2[:], in0=u2v, in1=w2b,
                                        op=OP.is_equal)
                kp = wpool.tile([P, TPB, 9], BF16, tag="kp")
                iokb = AP(ioksb.tensor, ioksb[:].offset,
                          [ioksb[:].ap[0], [0, TPB], [1, NCPC]])
                kpv = AP(kp.tensor, kp[:].offset,
                         [kp[:].ap[0], [9, TPB], [1, NCPC]])
                nc.gpsimd.tensor_tensor(out=kpv, in0=oh2[:], in1=iokb,
                                        op=OP.mult)
                # 8*outer (cc9 col 8) joins as the 9th reduce operand
                jv = AP(cc9.tensor, cc9[:].offset + 8,
                        [cc9[:].ap[0], [9, TPB]])
                jd = AP(kp.tensor, kp[:].offset + 8,
                        [kp[:].ap[0], [9, TPB]])
                nc.scalar.activation(jd, jv, AF.Copy)
                with nc.allow_low_precision(reason="ids are exact ints"):
                    nc.vector.tensor_reduce(
                        outbuf[:, b * TPB:(b + 1) * TPB], kp[:],
                        axis=AX.X, op=OP.add)

            nc.sync.dma_start(out[:], outbuf[:])
    nc.compile()
    return nc


def _prep_inputs(x, centers_outer, centers_inner):
    x = np.asarray(x, dtype=np.float32)
    co = np.asarray(centers_outer, dtype=np.float32)
    ci = np.asarray(centers_inner, dtype=np.float32)
    bf = ml_dtypes.bfloat16

    # stage-1 weights: u_j = 2*x1.c_j - |c_j|^2 + j*eps, bf16 hi/lo split;
    # bias triple-split so the eps tie-break survives bf16 rounding
    c2 = 2.0 * co.T                                   # [16, 64]
    ch = c2.astype(bf)
    cl = (c2 - ch.astype(np.float32)).astype(bf)
    b = -np.sum(co * co, axis=1) + EPS1 * np.arange(NC, dtype=np.float32)
    bh = b.astype(bf)
    br = b - bh.astype(np.float32)
    bl = br.astype(bf)
    bll = (br - bl.astype(np.float32)).astype(bf)
    w1 = np.zeros((35, 128), dtype=bf)
    w1[0:16, 0:64] = ch
    w1[16:32, 0:64] = ch
    w1[32, 0:64] = bh
    w1[33, 0:64] = bl
    w1[34, 0:64] = bll
    w1[0:16, 64:128] = cl

    # norm gather table [64, 9]: 8 negated norms + 8j id col, block-diag
    # duplicated for the paired gather
    tn = np.zeros((64, 9), dtype=np.float32)
    tn[:, 0:8] = -np.sum(ci * ci, axis=2)
    tn[:, 8] = 8.0 * np.arange(NC, dtype=np.float32)
    tabn = tn.astype(bf)

    # transposed coefficients [64 j, 128 (k,d)] = 2*ci[j,k,d],
    # duplicated on partitions 64:128 for odd-tile base alignment
    tabcT = np.tile((2.0 * ci.reshape(64, 128)).astype(bf), (2, 1))

    # fold pattern [128, 9]: F[(k,d), k'] = (k == k'); col 8 all-zero so
    # the fold matmul also closes the j-column's accumulation group
    foldF = np.zeros((128, 9), dtype=bf)
    foldF[:, 0:8] = np.repeat(np.eye(8, dtype=np.float32), 16, axis=0).astype(bf)

    ident = np.eye(128, dtype=bf)
    iotak = np.tile(np.arange(8, dtype=np.float32).astype(bf), (128, 1))

    in_maps = []
    for c in range(NCORES):
        xc = x[c * NPC:(c + 1) * NPC]
        if NPB > NPC:
            xc = np.concatenate(
                [xc, np.broadcast_to(xc[0], (NPB - NPC, 32))], axis=0)
        x1 = np.ascontiguousarray(xc[:, 0:16].T)      # [16, NPB]
        x1h = x1.astype(bf)
        x1l = (x1 - x1h.astype(np.float32)).astype(bf)
        xsc = np.empty((35, NPB), dtype=bf)
        xsc[0:16] = x1h
        xsc[16:32] = x1l
        xsc[32:35] = np.float32(1.0)
        # chunk-contiguous layouts: one dense read per XB-block group
        xsc = np.ascontiguousarray(
            xsc.reshape(35, NBX, XB * BS).transpose(1, 0, 2)
        ).reshape(NBX * 35, XB * BS)
        x2c = np.ascontiguousarray(xc[:, 16:32].T).astype(bf)  # [16, NPB]
        x2c8 = np.ascontiguousarray(
            np.broadcast_to(
                x2c.reshape(1, 16, NBX, XB * BS), (8, 16, NBX, XB * BS)
            ).transpose(2, 0, 1, 3)
        ).reshape(NBX * 128, XB * BS)

        in_maps.append({
            "xs": xsc, "x2r8": x2c8, "w1": w1, "tabn": tabn, "tabcT": tabcT,
            "foldF": foldF, "ident": ident, "iotak": iotak,
        })
    return in_maps


def _get_exec(nc):
    # mirror bass2jax.run_bass_via_pjrt but keep the jitted callable + device
    # inputs so repeated executions can be timed without re-transfer
    from concourse import bass2jax
    import concourse.mybir as _mb
    bass2jax.install_neuronx_cc_hook()
    partition_name = nc.partition_id_tensor.name if nc.partition_id_tensor else None
    in_names, out_names, out_avals, zero_outs = [], [], [], []
    for alloc in nc.m.functions[0].allocations:
        if not isinstance(alloc, _mb.MemoryLocationSet):
            continue
        name = alloc.memorylocations[0].name
        if alloc.kind == "ExternalInput":
            if name != partition_name:
                in_names.append(name)
        elif alloc.kind == "ExternalOutput":
            out_names.append(name)
            out_avals.append(jax.core.ShapedArray(
                tuple(alloc.tensor_shape), _mb.dt.np(alloc.dtype)))
            zero_outs.append(np.zeros(tuple(alloc.tensor_shape),
                                      _mb.dt.np(alloc.dtype)))
    n_params = len(in_names)
    in_names = in_names + out_names
    if partition_name is not None:
        in_names.append(partition_name)

    def _body(*args):
        operands = list(args)
        if partition_name is not None:
            operands.append(bass2jax.partition_id_tensor())
        outs = bass2jax._bass_exec_p.bind(
            *operands,
            out_avals=tuple(out_avals),
            in_names=tuple(in_names),
            out_names=tuple(out_names),
            lowering_input_output_aliases=(),
            sim_require_finite=True,
            sim_require_nnan=True,
            nc=nc,
        )
        return tuple(outs)

    devices = jax.devices()[:NCORES]
    mesh = Mesh(np.asarray(devices), ("core",))
    n_outs = len(out_names)
    sharded = jax.jit(
        shard_map(_body, mesh=mesh,
                  in_specs=(PartitionSpec("core"),) * (n_params + n_outs),
                  out_specs=(PartitionSpec("core"),) * n_outs,
                  check_rep=False),
        keep_unused=True,
    )
    return sharded, in_names[:n_params], out_names, out_avals, zero_outs, mesh


def _execute(in_maps, time_iters=0):
    if "nc" not in _cache:
        _cache["nc"] = _build_kernel()
        _cache["exec"] = _get_exec(_cache["nc"])
    sharded, in_names, out_names, out_avals, zero_outs, mesh = _cache["exec"]
    concat_in = [
        np.concatenate([m[name] for m in in_maps], axis=0) for name in in_names
    ]
    concat_zeros = [
        np.zeros((NCORES * z.shape[0], *z.shape[1:]), z.dtype) for z in zero_outs
    ]
    import time as _time
    from jax.sharding import NamedSharding
    shd = NamedSharding(mesh, PartitionSpec("core"))
    din = [jax.device_put(a, shd) for a in concat_in]
    dzero = [jax.device_put(z, shd) for z in concat_zeros]
    out_arrs = sharded(*din, *dzero)
    jax.block_until_ready(out_arrs)
    if time_iters:
        # amortized device-time measurement: a second NEFF repeats the whole
        # kernel K times in one dispatch; (t_K - t_1) / (K - 1) cancels the
        # dispatch/tunnel RTT that dominates single-dispatch wall time.
        K = 11
        if "exec_rep" not in _cache:
            _cache["nc_rep"] = _build_kernel(reps=K)
            _cache["exec_rep"] = _get_exec(_cache["nc_rep"])
        chain = _cache["exec_rep"][0]
        outs_c = chain(*din, *dzero)
        jax.block_until_ready(outs_c)
        t1s, tks = [], []
        for _ in range(max(time_iters, 8)):
            t0 = _time.perf_counter()
            out_arrs = sharded(*din, *dzero)
            jax.block_until_ready(out_arrs)
            t1s.append(_time.perf_counter() - t0)
            t0 = _time.perf_counter()
            outs_c = chain(*din, *dzero)
            jax.block_until_ready(outs_c)
            tks.append(_time.perf_counter() - t0)
        t1, tk = min(t1s), min(tks)
        _cache["exec_ns"] = int(max(tk - t1, 0.0) / (K - 1) * 1e9)
        _cache["dispatch_ns"] = int(t1 * 1e9)
    return [
        {name: np.asarray(out_arrs[i]).reshape(NCORES, *out_avals[i].shape)[c]
         for i, name in enumerate(out_names)}
        for c in range(NCORES)
    ]


def kernel(x, centers_outer, centers_inner):
    in_maps = _prep_inputs(x, centers_outer, centers_inner)
    results = _execute(in_maps, time_iters=3 if TRACE else 0)
    outs = []
    for c in range(NCORES):
        o = results[c]["out"]           # [128, NBLK*8] u16
        # sample 1024b + 128t + p sits at o[p, 8b + t]
        ids = o.reshape(P, NBLK, TPB).transpose(1, 2, 0).reshape(-1)[:NPC]
        outs.append(ids)
    return np.concatenate(outs).astype(np.int32)
